# revision 14
# baseline (speedup 1.0000x reference)
"""BitLinear158 Trainium2 kernel — fp8 DoubleRow with partial hi/lo correction.

Reference computation:
    gamma = mean(|W|)
    Wq    = clip(round(W / (gamma + 1e-5)), -1, 1)      # ternary {-1, 0, +1}
    out   = x @ Wq.T + b                                # x: [8, 4096, 2048]

Sharding: data-parallel over the batch dim (8 batches -> 8 cores). Each core
gets x[i] (host-transposed to k-major), the full W (host-transposed) and b.
gamma is computed redundantly per-core -- measured cross-core collective
latency/skew (~80us) far exceeds the 45us it would save.

Math: Wq is ternary so it is EXACT in fp8e4 (e4m3). The fp8 DoubleRow matmul
contracts K=256 per instruction at the same per-instruction cost as a bf16
K=128 matmul (measured ~216ns at 512 free rows) -> 2x FLOP rate. Activations
split x = hi + lo with hi = fp8(x), lo = fp8(x - hi); hi covers all 16
k-tiles, lo corrects k-tiles 8..15 (L=4 of 8 k-pairs). Output L2 rel error
= 2.35e-2 * sqrt(1 - L/8) -> 1.66e-2 measured on HW (gate 2e-2).

Ternarize (one op on each of GPSIMD and DVE, directly from W, exact fp32
compares -- a bf16 compare flips ~1e-3 of the weights and adds 4e-2 error):
    m  = 1{W <= thr}                (GPSIMD TensorTensor vs thr-filled tile)
    wq = 1{W >= -thr} - m           (DVE scalar_tensor_tensor IS_GE/SUBTRACT)
which is {-1, 0, +1} with reference-matching tie behavior.

Device pipeline per core:
  pass 1: stream WT (16 MiB, DMA-bound ~50us); |.|+row-sum partials split
          DVE/ACT; the last NRET W tiles stay resident in SBUF (pairs 7..3
          need no re-read). ones-matmul partition reduce; thresholds
          +-0.5*(gamma+eps); thr-filled tile via one ACT Identity op.
  pass 2: retained tiles quantize immediately (k-pairs 7..3 ready early,
          including all L lo-corrected pairs); tiles 5..0 re-stream
          descending and quantize on arrival.
  main:   epochs of 2 token-tiles x 4 output chunks = 8 concurrent
          [128,512] PSUM groups; per token tile: fp32 x DMA (deferred
          behind pass 1), ACT casts hi, GPSIMD computes lo = fp8(x - hi);
          12 DoubleRow matmuls per group in quantize-completion order;
          bias-add fused into the PSUM eviction on DVE; fp32 out. Final
          epoch emits group-major so evictions/DMA overlap the last matmuls.
"""

from contextlib import ExitStack

import numpy as np

import concourse.bacc as bacc
import concourse.bass as bass
import concourse.mybir as mybir
import concourse.tile as tile
from concourse.bass_utils import run_bass_kernel_spmd

P = 128
B, S, D_IN, D_OUT = 8, 4096, 2048, 2048
N_CORES = 8
TOK = (B * S) // N_CORES          # 4096 tokens per core
KT = D_IN // P                    # 16 k-tiles
KK = KT // 2                      # 8 k-pairs (DoubleRow contracts 2 tiles)
L = 4                             # k-pairs receiving the lo correction
TT = TOK // P                     # 32 token tiles
NC_CHUNK = 512                    # matmul moving free dim (1 PSUM bank fp32)
OC = D_OUT // NC_CHUNK            # 4 output chunks
W_ELEMS = D_OUT * D_IN            # 2**22 (power of 2: S/N == S*(1/N) exactly)
EPS = 1e-5
CKP0 = KK - L                     # first corrected k-pair (tiles 8..15)
NRET = 10                         # W tiles retained between pass 1 and quant
N_SGN = KT - NRET                 # re-streamed tiles use the sign-pair path

F32 = mybir.dt.float32
BF16 = mybir.dt.bfloat16
FP8 = mybir.dt.float8e4
DR = mybir.MatmulPerfMode.DoubleRow
MULT = mybir.AluOpType.mult
ADD = mybir.AluOpType.add
SUB = mybir.AluOpType.subtract
IS_GE = mybir.AluOpType.is_ge
IS_GT = mybir.AluOpType.is_gt
AX_X = mybir.AxisListType.X


def build_nc() -> bass.Bass:
    nc = bacc.Bacc(None, target_bir_lowering=False)
    xT = nc.dram_tensor("xT", [D_IN, TOK], F32, kind="ExternalInput")
    WT = nc.dram_tensor("WT", [D_IN, D_OUT], F32, kind="ExternalInput")
    b = nc.dram_tensor("b", [D_OUT], F32, kind="ExternalInput")
    out = nc.dram_tensor("out", [TOK, D_OUT], F32, kind="ExternalOutput")

    with tile.TileContext(nc) as tc, ExitStack() as ctx:
        wpool = ctx.enter_context(tc.tile_pool(name="wpass", bufs=NRET + 1))
        spool = ctx.enter_context(tc.tile_pool(name="scalars", bufs=1))
        mpool = ctx.enter_context(tc.tile_pool(name="mle", bufs=4))
        wqpool = ctx.enter_context(tc.tile_pool(name="wq", bufs=1))
        xfpool = ctx.enter_context(tc.tile_pool(name="xf", bufs=3))
        xhpool = ctx.enter_context(tc.tile_pool(name="xh", bufs=5))
        xlpool = ctx.enter_context(tc.tile_pool(name="xl", bufs=5))
        opool = ctx.enter_context(tc.tile_pool(name="osb", bufs=2))
        pspool = ctx.enter_context(
            tc.tile_pool(name="psum", bufs=8, space="PSUM")
        )

        xT_v = xT.rearrange("(a p) t -> p a t", p=P)  # [128, KT, TOK]
        xhs, xls = {}, {}
        xfs = {}
        first_xf_dma = [True]

        def emit_xf(tt):
            xf = xfpool.tile([P, KT, P], F32, tag="xf")
            xf_dma = nc.gpsimd.dma_start(
                xf[:], xT_v[:, :, tt * P : (tt + 1) * P]
            )
            if first_xf_dma[0]:
                # x competes with the gamma-critical W stream for HBM;
                # hold it back until pass 1 is issued.
                first_xf_dma[0] = False
                tile.add_dep_helper(
                    xf_dma.ins, last_w1_dma.ins, reason="defer x behind pass1"
                )
            xfs[tt] = xf

        def emit_hilo(tt):
            xf = xfs.pop(tt)
            xh = xhpool.tile([P, KT, P], FP8, tag="xh")
            # tiles 0..N_SGN-1 carry {-2,0,2} weights; halve their x here
            # (exact exponent shift) so the products match {-1,0,1}.
            nc.scalar.activation(
                xh[:, :N_SGN, :],
                xf[:, :N_SGN, :],
                mybir.ActivationFunctionType.Copy,
                scale=0.5,
            )
            nc.scalar.activation(
                xh[:, N_SGN:, :], xf[:, N_SGN:, :],
                mybir.ActivationFunctionType.Copy,
            )
            xl = xlpool.tile([P, 2 * L, P], FP8, tag="xl")
            nc.gpsimd.tensor_sub(
                xl[:], xf[:, 2 * CKP0 :, :], xh[:, 2 * CKP0 :, :]
            )
            xhs[tt], xls[tt] = xh, xl

        # ---- pass 1: gamma = mean |W|; |.|+row-sum split DVE/ACT so the
        # pass is DMA-bound. The last NRET tiles stay resident.
        partials_dve = spool.tile([P, KT // 2], F32)
        partials_act = spool.tile([P, KT // 2], F32)
        dump = spool.tile([P, D_OUT], BF16)
        w_resident = {}
        last_w1_dma = None
        for kt in range(KT):
            wt = wpool.tile([P, D_OUT], F32, tag="wt", name=f"w1_{kt}")
            last_w1_dma = nc.sync.dma_start(wt[:], WT[kt * P : (kt + 1) * P, :])
            if kt % 2 == 0:
                nc.vector.reduce_sum(
                    partials_dve[:, kt // 2 : kt // 2 + 1],
                    wt[:],
                    axis=AX_X,
                    apply_absolute_value=True,
                )
            else:
                nc.scalar.activation(
                    dump[:],
                    wt[:],
                    mybir.ActivationFunctionType.Abs,
                    accum_out=partials_act[:, kt // 2 : kt // 2 + 1],
                )
            if kt >= KT - NRET:
                w_resident[kt] = wt

        c1 = spool.tile([P, 1], F32)
        nc.vector.reduce_sum(c1[:], partials_dve[:], axis=AX_X)
        c2 = spool.tile([P, 1], F32)
        nc.vector.reduce_sum(c2[:], partials_act[:], axis=AX_X)
        colsum = spool.tile([P, 1], F32)
        nc.vector.tensor_add(colsum[:], c1[:], c2[:])

        # Partition reduce + broadcast in one PE op.
        ones_sq = spool.tile([P, P], F32)
        nc.vector.memset(ones_sq[:], 1.0)
        total_ps = pspool.tile([P, NC_CHUNK], F32, tag="ps")
        nc.tensor.matmul(
            total_ps[:, 0:1], ones_sq[:], colsum[:], start=True, stop=True
        )

        # Thresholds: Wq = 1{W >= -thr} - 1{W <= thr},  thr = 0.5*(gamma+eps)
        geps = spool.tile([P, 1], F32)
        nc.vector.tensor_scalar(
            geps[:], total_ps[:, 0:1], 1.0 / W_ELEMS, EPS, MULT, ADD
        )
        thr = spool.tile([P, 1], F32)
        nc.vector.tensor_scalar_mul(thr[:], geps[:], 0.5)
        negthr = spool.tile([P, 1], F32)
        nc.vector.tensor_scalar_mul(negthr[:], geps[:], -0.5)

        # ---- pass 2: quantize ----
        # Retained tiles (DVE, 2 ops, {-1,0,+1}):
        #   ga = (W > thr) - 1;  wq = (W >= -thr) + ga
        # Re-streamed tiles (ACT 2x Sign + GPSIMD add, {-2,0,+2}):
        #   wq2 = Sign(W - thr) + Sign(W + thr)
        # The x-cast halves the activations of the re-streamed k-range so
        # the 2x weight scale cancels exactly (0.5x is an exponent shift,
        # exact in fp8).
        wq8 = wqpool.tile([P, KT, D_OUT], FP8)

        def emit_quant_dve(kt, wt):
            ga = mpool.tile([P, D_OUT], FP8, tag="m", name=f"ga{kt}")
            nc.vector.tensor_scalar(ga[:], wt[:], thr[:], -1.0, IS_GT, ADD)
            nc.vector.scalar_tensor_tensor(
                wq8[:, kt, :], wt[:], negthr[:], ga[:], IS_GE, ADD
            )

        def emit_quant_sgn(kt, wt):
            a = mpool.tile([P, D_OUT], FP8, tag="m", name=f"a{kt}")
            nc.scalar.sign(a[:], wt[:], bias=negthr[:])
            c = mpool.tile([P, D_OUT], FP8, tag="m", name=f"c{kt}")
            nc.scalar.sign(c[:], wt[:], bias=thr[:])
            nc.gpsimd.tensor_tensor(wq8[:, kt, :], a[:], c[:], ADD)

        # ep0's x DMAs and hi/lo first so the first matmuls' stationaries
        # are ready earliest; then retained quantize on DVE (descending);
        # re-streamed tiles quantize on ACT+GPSIMD as they arrive.
        emit_xf(0)
        emit_xf(1)
        emit_hilo(0)
        emit_hilo(1)
        bias_sb = spool.tile([P, D_OUT], F32)
        b_row = b[:].rearrange("(o d) -> o d", o=1)
        for j, kt in enumerate(range(KT - NRET - 1, -1, -1)):
            wt = wpool.tile([P, D_OUT], F32, tag="wt", name=f"w2_{kt}")
            nc.sync.dma_start(wt[:], WT[kt * P : (kt + 1) * P, :])
            emit_quant_sgn(kt, wt)
            if j == 1:
                nc.sync.dma_start(
                    bias_sb[:], b_row.to_broadcast((P, D_OUT))
                )
        for kt in range(KT - 1, KT - NRET - 1, -1):
            emit_quant_dve(kt, w_resident[kt])

        # Per-group matmul emission order (quantize-completion order):
        # DVE-retained pairs 7..4 (lo right after hi), ACT-path 2,1,0,
        # pair 3 (retained tiles 6,7, quantized last on DVE) closes.
        MM_ORDER = (
            [("h", 7), ("l", 7), ("h", 6), ("l", 6)]
            + [("h", 5), ("l", 5), ("h", 4), ("l", 4)]
            + [("h", 2), ("h", 1), ("h", 0), ("h", 3)]
        )

        # ---- main: out[t, :] = x[t, :] @ WqT + b ----
        TPE = 2  # token tiles per epoch
        NEP = TT // TPE
        for ep in range(NEP):
            for i in range(TPE):
                tt = ep * TPE + i
                if tt not in xhs:
                    emit_xf(tt)
                    emit_hilo(tt)

            groups = [(i, oc) for i in range(TPE) for oc in range(OC)]
            pss = [
                pspool.tile([P, NC_CHUNK], F32, tag="ps", name=f"ps{g}")
                for g in range(len(groups))
            ]

            def emit_mm(g, mi):
                i, oc = groups[g]
                kind, kkp = MM_ORDER[mi]
                tt = ep * TPE + i
                if kind == "h":
                    stat = xhs[tt][:, 2 * kkp : 2 * kkp + 2, :]
                else:
                    stat = xls[tt][:, 2 * (kkp - CKP0) : 2 * (kkp - CKP0) + 2, :]
                nc.tensor.matmul(
                    pss[g][:],
                    stat,
                    wq8[:, 2 * kkp : 2 * kkp + 2,
                        oc * NC_CHUNK : (oc + 1) * NC_CHUNK],
                    start=(mi == 0),
                    stop=(mi == len(MM_ORDER) - 1),
                    perf_mode=DR,
                )

            if ep < NEP - 1:
                for mi in range(len(MM_ORDER)):
                    for g in range(len(groups)):
                        emit_mm(g, mi)
            else:
                # final epoch: group-major so early groups' evictions and
                # out-DMA overlap the remaining groups' matmuls.
                for g in range(len(groups)):
                    for mi in range(len(MM_ORDER)):
                        emit_mm(g, mi)

            for i in range(TPE):
                tt = ep * TPE + i
                osb = opool.tile([P, D_OUT], F32, tag="osb")
                for oc in range(OC):
                    nc.vector.tensor_add(
                        osb[:, oc * NC_CHUNK : (oc + 1) * NC_CHUNK],
                        pss[i * OC + oc][:],
                        bias_sb[:, oc * NC_CHUNK : (oc + 1) * NC_CHUNK],
                    )
                nc.sync.dma_start(out[tt * P : (tt + 1) * P, :], osb[:])
                del xhs[tt], xls[tt]

    nc.finalize()
    return nc


_NC_CACHE: list = []


def _get_nc() -> bass.Bass:
    if not _NC_CACHE:
        _NC_CACHE.append(build_nc())
    return _NC_CACHE[0]


def make_in_maps(x: np.ndarray, W: np.ndarray, b: np.ndarray):
    x = np.asarray(x, dtype=np.float32).reshape(N_CORES, TOK, D_IN)
    W = np.asarray(W, dtype=np.float32)
    b = np.asarray(b, dtype=np.float32)
    WT = np.ascontiguousarray(W.T)
    return [
        {"xT": np.ascontiguousarray(x[c].T), "WT": WT, "b": b}
        for c in range(N_CORES)
    ]


def run(x, W, b, **spmd_kwargs):
    """Run the SPMD kernel; returns (full_output, BassKernelResults)."""
    nc = _get_nc()
    in_maps = make_in_maps(x, W, b)
    res = run_bass_kernel_spmd(nc, in_maps, list(range(N_CORES)), **spmd_kwargs)
    out = np.stack([res.results[c]["out"] for c in range(N_CORES)], axis=0)
    return out.reshape(B, S, D_OUT), res


def kernel(x, W, b):
    out, _ = run(x, W, b)
    return out
